# revision 14
# baseline (speedup 1.0000x reference)
"""Multi-head attention (B=4, S=2048, D=2048, H=16) on 8 trn2 NeuronCores.

Sharding: tensor-parallel over heads — 2 heads per core. Each core computes
its heads' Q/K/V projections, full attention for those heads, and a partial
output projection (its 256 rows of wo). The host sums the 8 partial outputs.

On-core layout: everything is kept "feature-major" ([d, token]) so that the
contraction dim always lands on SBUF partitions:
  - host ships xT [D, B*S] (tf32-pre-rounded, fp32r matmuls run at full PE rate)
  - QT/KT [128, tokens] per head come straight out of the projection matmuls
  - scores are computed transposed ([k, q]); exp is fused over two key-chunks
    per ACT instruction; the softmax denominator is a DVE tile-sum + one
    ones-matmul partition reduce; normalization is folded into the PSUM->SBUF
    copy of the unnormalized attention output (flash-style, no max needed
    since scores are ~N(0,1) in fp32).
  - the partial out-projection is interleaved per query-span so the PE has
    dense work while ACT works through the exps.
"""
import os
import sys

sys.path.insert(0, "/opt/trn_rl_repo")
import numpy as np

B, S, D, H = 4, 2048, 2048, 16
HD = 128
NCORES = 8
HP = H // NCORES          # heads per core = 2
DC = HP * HD              # per-core slice of D = 256
TOK = B * S               # 8192
SCALE = HD ** -0.5
NDC = D // 128            # 16 contraction chunks for the projections
SPAN = 256                # token span per projection step
NSPAN = S // SPAN         # 8 spans per batch
QS = 512                  # query span in attention
NQS = S // QS             # 4
NKC = S // 128            # 16 key chunks

LAST_EXEC_NS = None
_BUILT = None


def _round_tf32(x: np.ndarray) -> np.ndarray:
    """Round fp32 to tf32 (10 mantissa bits, RNE), keep fp32 container."""
    u = np.ascontiguousarray(x, dtype=np.float32).view(np.uint32)
    bias = np.uint32(0x00000FFF) + ((u >> np.uint32(13)) & np.uint32(1))
    return ((u + bias) & np.uint32(0xFFFFE000)).view(np.float32)


def _build():
    global _BUILT
    if _BUILT is not None:
        return _BUILT
    import concourse.tile as tile
    from concourse import bacc, mybir

    F32R = mybir.dt.float32r
    F32 = mybir.dt.float32
    Exp = mybir.ActivationFunctionType.Exp
    Ident = mybir.ActivationFunctionType.Identity

    nc = bacc.Bacc("TRN2", target_bir_lowering=False, debug=False)
    xt = nc.dram_tensor("xt", [D, TOK], F32R, kind="ExternalInput")
    wq = nc.dram_tensor("wq", [D, DC], F32R, kind="ExternalInput")
    wk = nc.dram_tensor("wk", [D, DC], F32R, kind="ExternalInput")
    wv = nc.dram_tensor("wv", [D, DC], F32R, kind="ExternalInput")
    wo = nc.dram_tensor("wo", [DC, D], F32R, kind="ExternalInput")
    bq2 = nc.dram_tensor("bq2", [HD, HP], F32, kind="ExternalInput")
    bk2 = nc.dram_tensor("bk2", [HD, HP], F32, kind="ExternalInput")
    ones = nc.dram_tensor("ones", [128, 128], F32R, kind="ExternalInput")
    out = nc.dram_tensor("out", [TOK, D], F32, kind="ExternalOutput")

    with tile.TileContext(nc) as tc:
        with tc.tile_pool(name="const", bufs=1) as cpool, \
             tc.tile_pool(name="xp", bufs=2) as xpool, \
             tc.tile_pool(name="bt", bufs=1) as bpool, \
             tc.tile_pool(name="at", bufs=3) as apool, \
             tc.tile_pool(name="ot", bufs=2) as opool, \
             tc.tile_pool(name="ps", bufs=1, space="PSUM") as ps:

            wq_sb = cpool.tile([128, NDC, DC], F32R)
            wk_sb = cpool.tile([128, NDC, DC], F32R)
            wv_sb = cpool.tile([128, NDC, DC], F32R)
            wo_sb = cpool.tile([128, HP, D], F32R)
            ones_sb = cpool.tile([128, 128], F32R)
            bq_sb = cpool.tile([HD, HP], F32)
            bk_sb = cpool.tile([HD, HP], F32)
            nc.sync.dma_start(out=wq_sb, in_=wq.rearrange("(c p) n -> p c n", p=128))
            nc.sync.dma_start(out=bq_sb, in_=bq2[:, :])
            nc.sync.dma_start(out=bk_sb, in_=bk2[:, :])

            xt_r = xt.rearrange("(c p) t -> p c t", p=128)

            for b in range(B):
                # ---- A) Q/K/V projections for batch b ----
                qt_b = bpool.tile([128, HP, S], F32R, name="qt_b", tag="qt_b")
                kt_b = bpool.tile([128, HP, S], F32R, name="kt_b", tag="kt_b")
                v_b = bpool.tile([128, NKC, DC], F32R, name="v_b", tag="v_b")
                for sp in range(NSPAN):
                    t0 = b * S + sp * SPAN
                    xsp = xpool.tile([128, NDC, SPAN], F32R, name="xsp", tag="xsp")
                    nc.sync.dma_start(out=xsp, in_=xt_r[:, :, t0:t0 + SPAN])
                    if b == 0 and sp == 0:
                        # wk/wv queue behind wq + the first x span so the PE
                        # can start the Q projection as early as possible
                        nc.sync.dma_start(
                            out=wk_sb, in_=wk.rearrange("(c p) n -> p c n", p=128))
                        nc.sync.dma_start(
                            out=wv_sb, in_=wv.rearrange("(c p) n -> p c n", p=128))
                    for h in range(HP):
                        # Q and K accumulate into halves of one PSUM bank
                        qkps = ps.tile([128, 2 * SPAN], F32, name="qkps",
                                       tag="pj", bufs=2)
                        for c in range(NDC):
                            nc.tensor.matmul(
                                qkps[:, 0:SPAN], wq_sb[:, c, h * HD:(h + 1) * HD],
                                xsp[:, c, :], start=(c == 0), stop=(c == NDC - 1))
                        for c in range(NDC):
                            nc.tensor.matmul(
                                qkps[:, SPAN:2 * SPAN],
                                wk_sb[:, c, h * HD:(h + 1) * HD],
                                xsp[:, c, :], start=(c == 0), stop=(c == NDC - 1))
                        nc.scalar.activation(
                            qt_b[:, h, sp * SPAN:(sp + 1) * SPAN],
                            qkps[:, 0:SPAN], Ident, bias=bq_sb[:, h:h + 1])
                        nc.scalar.activation(
                            kt_b[:, h, sp * SPAN:(sp + 1) * SPAN],
                            qkps[:, SPAN:2 * SPAN], Ident,
                            bias=bk_sb[:, h:h + 1])
                    # both V token-chunks accumulate into one PSUM bank
                    vps = ps.tile([128, 2 * DC], F32, name="vps", tag="pj",
                                  bufs=2)
                    for tch in range(SPAN // 128):
                        for c in range(NDC):
                            nc.tensor.matmul(
                                vps[:, tch * DC:(tch + 1) * DC],
                                xsp[:, c, tch * 128:(tch + 1) * 128],
                                wv_sb[:, c, :], start=(c == 0), stop=(c == NDC - 1))
                    for tch in range(SPAN // 128):
                        nc.scalar.copy(
                            v_b[:, sp * (SPAN // 128) + tch, :],
                            vps[:, tch * DC:(tch + 1) * DC])

                if b == 0:
                    # deferred so batch-0 x spans win the DMA queue at startup
                    nc.sync.dma_start(
                        out=wo_sb, in_=wo.rearrange("(c p) n -> p c n", p=128))
                    nc.sync.dma_start(out=ones_sb, in_=ones[:, :])

                # ---- B) attention + interleaved partial out-projection ----
                avt_b = bpool.tile([128, HP, S], F32R, name="avt_b", tag="avt_b")
                for qs in range(NQS):
                    for h in range(HP):
                        q_sl = qt_b[:, h, qs * QS:(qs + 1) * QS]
                        av_ps = ps.tile([HD, QS], F32, name="av_ps",
                                        tag="acc", bufs=2)
                        dn_ps = ps.tile([128, QS], F32, name="dn_ps",
                                        tag="acc", bufs=2)

                        def emit_av(kp, p_prev):
                            # AV and the softmax-denominator ones-matmul both
                            # consume the exp tile on the PE — keeps the PE
                            # dense (no DVE/GPSIMD reduction chains, no HAM
                            # cool-down gaps)
                            for j in range(2):
                                kc = 2 * kp + j
                                nc.tensor.matmul(
                                    av_ps, v_b[:, kc, h * HD:(h + 1) * HD],
                                    p_prev[:, j * QS:(j + 1) * QS],
                                    start=(kc == 0), stop=(kc == NKC - 1))
                            for j in range(2):
                                kc = 2 * kp + j
                                nc.tensor.matmul(
                                    dn_ps, ones_sb,
                                    p_prev[:, j * QS:(j + 1) * QS],
                                    start=(kc == 0), stop=(kc == NKC - 1))

                        p_prev = None
                        for kp in range(NKC // 2):
                            # two key-chunks share one psum tile and one exp;
                            # AV of pair kp-1 is emitted after the scores of
                            # pair kp so the PE never heads-of-line blocks on
                            # the exp it needs
                            s_ps = ps.tile([128, 2 * QS], F32, name="s_ps",
                                           tag="s", bufs=2)
                            p_sb = apool.tile([128, 2 * QS], F32R, name="p_sb",
                                              tag="p", bufs=4)
                            for j in range(2):
                                kc = 2 * kp + j
                                nc.tensor.matmul(
                                    s_ps[:, j * QS:(j + 1) * QS],
                                    kt_b[:, h, kc * 128:(kc + 1) * 128], q_sl,
                                    start=True, stop=True)
                            nc.scalar.activation(p_sb, s_ps, Exp, scale=SCALE)
                            if p_prev is not None:
                                emit_av(kp - 1, p_prev)
                            p_prev = p_sb
                        emit_av(NKC // 2 - 1, p_prev)
                        recip = apool.tile([128, QS], F32, name="recip",
                                           tag="recip", bufs=1)
                        nc.vector.reciprocal_approx_fast(recip, dn_ps)
                        nc.vector.tensor_mul(
                            avt_b[:, h, qs * QS:(qs + 1) * QS], av_ps, recip)

                    # partial out-projection for this query span (both heads
                    # are now done for tokens qs*QS .. (qs+1)*QS)
                    for tloc in range(QS // 128):
                        tch = qs * (QS // 128) + tloc
                        out_sb = opool.tile([128, D], F32, name="out_sb",
                                            tag="out_sb")
                        for dsp in range(D // 512):
                            ops = ps.tile([128, 512], F32, name="ops", tag="pj",
                                          bufs=2)
                            for h in range(HP):
                                nc.tensor.matmul(
                                    ops, avt_b[:, h, tch * 128:(tch + 1) * 128],
                                    wo_sb[:, h, dsp * 512:(dsp + 1) * 512],
                                    start=(h == 0), stop=(h == HP - 1))
                            nc.vector.tensor_copy(
                                out_sb[:, dsp * 512:(dsp + 1) * 512], ops)
                        nc.sync.dma_start(
                            out=out[b * S + tch * 128:b * S + (tch + 1) * 128, :],
                            in_=out_sb)
    nc.compile()
    _BUILT = nc
    return nc


def _install_trace_hooks():
    import types
    try:
        import antenv.axon_hooks  # noqa: F401
        return True
    except ImportError:
        pass
    try:
        from trn_agent_boot.trn_boot import _ntff_profile_via_ctypes
        hook = _ntff_profile_via_ctypes('/opt/axon/libaxon_pjrt.so')
        if hook is None:
            return False
        m = types.ModuleType('antenv.axon_hooks')
        m.get_axon_ntff_profile_hook = lambda: hook
        sys.modules['antenv.axon_hooks'] = m
        from concourse import bass_utils
        bass_utils.upload_artifacts = lambda tmpdir: "local://" + tmpdir
        return True
    except Exception:
        return False


def kernel(x, wq, bq, wk, bk, wv, bv, wo, bo):
    global LAST_EXEC_NS
    from concourse.bass_utils import run_bass_kernel_spmd

    x = np.asarray(x, dtype=np.float32)
    wq = np.asarray(wq, dtype=np.float32)
    bq = np.asarray(bq, dtype=np.float32)
    wk = np.asarray(wk, dtype=np.float32)
    bk = np.asarray(bk, dtype=np.float32)
    wv = np.asarray(wv, dtype=np.float32)
    bv = np.asarray(bv, dtype=np.float32)
    wo = np.asarray(wo, dtype=np.float32)
    bo = np.asarray(bo, dtype=np.float32)

    xt = _round_tf32(x.reshape(TOK, D).T)
    ones = np.ones((128, 128), dtype=np.float32)
    in_maps = []
    for i in range(NCORES):
        sl = slice(i * DC, (i + 1) * DC)
        in_maps.append({
            "xt": xt,
            "wq": _round_tf32(wq[:, sl]),
            "wk": _round_tf32(wk[:, sl]),
            "wv": _round_tf32(wv[:, sl]),
            "wo": _round_tf32(wo[sl, :]),
            "bq2": np.ascontiguousarray(bq[sl].reshape(HP, HD).T),
            "bk2": np.ascontiguousarray(bk[sl].reshape(HP, HD).T),
            "ones": ones,
        })

    trace = bool(os.environ.get("KERNEL_TRACE"))
    if trace:
        trace = _install_trace_hooks()

    nc = _build()
    res = run_bass_kernel_spmd(nc, in_maps, list(range(NCORES)), trace=trace)
    LAST_EXEC_NS = res.exec_time_ns

    total = np.zeros((TOK, D), dtype=np.float32)
    for r in res.results:
        total += r["out"]
    # V-bias folds into a constant row: softmax rows sum to 1, so
    # attention(V + 1*bv^T) = attention(V) + 1*bv^T, and (bv @ wo) adds to bo.
    total += bo + bv @ wo
    return total.reshape(B, S, D)


# revision 16
# speedup vs baseline: 1.0284x; 1.0284x over previous
"""Multi-head attention (B=4, S=2048, D=2048, H=16) on 8 trn2 NeuronCores.

Sharding: tensor-parallel over heads — 2 heads per core. Each core computes
its heads' Q/K/V projections, full attention for those heads, and a partial
output projection (its 256 rows of wo). The host sums the 8 partial outputs.

On-core layout: everything is kept "feature-major" ([d, token]) so that the
contraction dim always lands on SBUF partitions:
  - host ships xT [D, B*S] (tf32-pre-rounded, fp32r matmuls run at full PE rate)
  - QT/KT [128, tokens] per head come straight out of the projection matmuls
  - scores are computed transposed ([k, q]); exp is fused over two key-chunks
    per ACT instruction; the softmax denominator is a DVE tile-sum + one
    ones-matmul partition reduce; normalization is folded into the PSUM->SBUF
    copy of the unnormalized attention output (flash-style, no max needed
    since scores are ~N(0,1) in fp32).
  - the partial out-projection is interleaved per query-span so the PE has
    dense work while ACT works through the exps.
"""
import os
import sys

sys.path.insert(0, "/opt/trn_rl_repo")
import numpy as np

B, S, D, H = 4, 2048, 2048, 16
HD = 128
NCORES = 8
HP = H // NCORES          # heads per core = 2
DC = HP * HD              # per-core slice of D = 256
TOK = B * S               # 8192
SCALE = HD ** -0.5
NDC = D // 128            # 16 contraction chunks for the projections
SPAN = 256                # token span per projection step
NSPAN = S // SPAN         # 8 spans per batch
QS = 512                  # query span in attention
NQS = S // QS             # 4
NKC = S // 128            # 16 key chunks

LAST_EXEC_NS = None
_BUILT = None


def _round_tf32(x: np.ndarray) -> np.ndarray:
    """Round fp32 to tf32 (10 mantissa bits, RNE), keep fp32 container."""
    u = np.ascontiguousarray(x, dtype=np.float32).view(np.uint32)
    bias = np.uint32(0x00000FFF) + ((u >> np.uint32(13)) & np.uint32(1))
    return ((u + bias) & np.uint32(0xFFFFE000)).view(np.float32)


def _build():
    global _BUILT
    if _BUILT is not None:
        return _BUILT
    import concourse.tile as tile
    from concourse import bacc, mybir

    F32R = mybir.dt.float32r
    F32 = mybir.dt.float32
    Exp = mybir.ActivationFunctionType.Exp
    Ident = mybir.ActivationFunctionType.Identity

    nc = bacc.Bacc("TRN2", target_bir_lowering=False, debug=False)
    xt = nc.dram_tensor("xt", [D, TOK], F32R, kind="ExternalInput")
    wq = nc.dram_tensor("wq", [D, DC], F32R, kind="ExternalInput")
    wk = nc.dram_tensor("wk", [D, DC], F32R, kind="ExternalInput")
    wv = nc.dram_tensor("wv", [D, DC], F32R, kind="ExternalInput")
    wo = nc.dram_tensor("wo", [DC, D], F32R, kind="ExternalInput")
    bq2 = nc.dram_tensor("bq2", [HD, HP], F32, kind="ExternalInput")
    bk2 = nc.dram_tensor("bk2", [HD, HP], F32, kind="ExternalInput")
    ones = nc.dram_tensor("ones", [128, 128], F32R, kind="ExternalInput")
    out = nc.dram_tensor("out", [TOK, D], F32, kind="ExternalOutput")

    with tile.TileContext(nc) as tc:
        with tc.tile_pool(name="const", bufs=1) as cpool, \
             tc.tile_pool(name="xp", bufs=3) as xpool, \
             tc.tile_pool(name="bt", bufs=1) as bpool, \
             tc.tile_pool(name="at", bufs=3) as apool, \
             tc.tile_pool(name="ot", bufs=2) as opool, \
             tc.tile_pool(name="ps", bufs=1, space="PSUM") as ps:

            wq_sb = cpool.tile([128, NDC, DC], F32R)
            wk_sb = cpool.tile([128, NDC, DC], F32R)
            wv_sb = cpool.tile([128, NDC, DC], F32R)
            wo_sb = cpool.tile([128, HP, D], F32R)
            ones_sb = cpool.tile([128, 128], F32R)
            bq_sb = cpool.tile([HD, HP], F32)
            bk_sb = cpool.tile([HD, HP], F32)
            nc.sync.dma_start(out=wq_sb, in_=wq.rearrange("(c p) n -> p c n", p=128))
            nc.sync.dma_start(out=bq_sb, in_=bq2[:, :])
            nc.sync.dma_start(out=bk_sb, in_=bk2[:, :])

            xt_r = xt.rearrange("(c p) t -> p c t", p=128)

            for b in range(B):
                # ---- A) Q/K/V projections for batch b ----
                qt_b = bpool.tile([128, HP, S], F32R, name="qt_b", tag="qt_b")
                kt_b = bpool.tile([128, HP, S], F32R, name="kt_b", tag="kt_b")
                v_b = bpool.tile([128, NKC, DC], F32R, name="v_b", tag="v_b")
                for sp in range(NSPAN):
                    t0 = b * S + sp * SPAN
                    xsp = xpool.tile([128, NDC, SPAN], F32R, name="xsp", tag="xsp")
                    nc.sync.dma_start(out=xsp, in_=xt_r[:, :, t0:t0 + SPAN])
                    if b == 0 and sp == 0:
                        # wk/wv go on the scalar HWDGE ring so they stream in
                        # parallel with wq + the first x span on the sync ring
                        nc.scalar.dma_start(
                            out=wk_sb, in_=wk.rearrange("(c p) n -> p c n", p=128))
                        nc.scalar.dma_start(
                            out=wv_sb, in_=wv.rearrange("(c p) n -> p c n", p=128))
                    for h in range(HP):
                        # Q and K accumulate into halves of one PSUM bank
                        qkps = ps.tile([128, 2 * SPAN], F32, name="qkps",
                                       tag="pj", bufs=2)
                        for c in range(NDC):
                            nc.tensor.matmul(
                                qkps[:, 0:SPAN], wq_sb[:, c, h * HD:(h + 1) * HD],
                                xsp[:, c, :], start=(c == 0), stop=(c == NDC - 1))
                        for c in range(NDC):
                            nc.tensor.matmul(
                                qkps[:, SPAN:2 * SPAN],
                                wk_sb[:, c, h * HD:(h + 1) * HD],
                                xsp[:, c, :], start=(c == 0), stop=(c == NDC - 1))
                        nc.scalar.activation(
                            qt_b[:, h, sp * SPAN:(sp + 1) * SPAN],
                            qkps[:, 0:SPAN], Ident, bias=bq_sb[:, h:h + 1])
                        nc.scalar.activation(
                            kt_b[:, h, sp * SPAN:(sp + 1) * SPAN],
                            qkps[:, SPAN:2 * SPAN], Ident,
                            bias=bk_sb[:, h:h + 1])
                    # both V token-chunks accumulate into one PSUM bank
                    vps = ps.tile([128, 2 * DC], F32, name="vps", tag="pj",
                                  bufs=2)
                    for tch in range(SPAN // 128):
                        for c in range(NDC):
                            nc.tensor.matmul(
                                vps[:, tch * DC:(tch + 1) * DC],
                                xsp[:, c, tch * 128:(tch + 1) * 128],
                                wv_sb[:, c, :], start=(c == 0), stop=(c == NDC - 1))
                    for tch in range(SPAN // 128):
                        nc.scalar.copy(
                            v_b[:, sp * (SPAN // 128) + tch, :],
                            vps[:, tch * DC:(tch + 1) * DC])

                if b == 0:
                    # deferred so batch-0 x spans win the DMA queue at startup
                    nc.sync.dma_start(
                        out=wo_sb, in_=wo.rearrange("(c p) n -> p c n", p=128))
                    nc.sync.dma_start(out=ones_sb, in_=ones[:, :])

                # ---- B) attention + interleaved partial out-projection ----
                avt_b = bpool.tile([128, HP, S], F32R, name="avt_b", tag="avt_b")
                for qs in range(NQS):
                    for h in range(HP):
                        q_sl = qt_b[:, h, qs * QS:(qs + 1) * QS]
                        av_ps = ps.tile([HD, QS], F32, name="av_ps",
                                        tag="acc", bufs=2)
                        dn_ps = ps.tile([128, QS], F32, name="dn_ps",
                                        tag="acc", bufs=2)

                        def emit_av(kp, p_prev):
                            # AV and the softmax-denominator ones-matmul both
                            # consume the exp tile on the PE — keeps the PE
                            # dense (no DVE/GPSIMD reduction chains, no HAM
                            # cool-down gaps)
                            for j in range(2):
                                kc = 2 * kp + j
                                nc.tensor.matmul(
                                    av_ps, v_b[:, kc, h * HD:(h + 1) * HD],
                                    p_prev[:, j * QS:(j + 1) * QS],
                                    start=(kc == 0), stop=(kc == NKC - 1))
                            for j in range(2):
                                kc = 2 * kp + j
                                nc.tensor.matmul(
                                    dn_ps, ones_sb,
                                    p_prev[:, j * QS:(j + 1) * QS],
                                    start=(kc == 0), stop=(kc == NKC - 1))

                        p_prev = None
                        for kp in range(NKC // 2):
                            # two key-chunks share one psum tile and one exp;
                            # AV of pair kp-1 is emitted after the scores of
                            # pair kp so the PE never heads-of-line blocks on
                            # the exp it needs
                            s_ps = ps.tile([128, 2 * QS], F32, name="s_ps",
                                           tag="s", bufs=2)
                            p_sb = apool.tile([128, 2 * QS], F32R, name="p_sb",
                                              tag="p", bufs=3)
                            for j in range(2):
                                kc = 2 * kp + j
                                nc.tensor.matmul(
                                    s_ps[:, j * QS:(j + 1) * QS],
                                    kt_b[:, h, kc * 128:(kc + 1) * 128], q_sl,
                                    start=True, stop=True)
                            nc.scalar.activation(p_sb, s_ps, Exp, scale=SCALE)
                            if p_prev is not None:
                                emit_av(kp - 1, p_prev)
                            p_prev = p_sb
                        emit_av(NKC // 2 - 1, p_prev)
                        recip = apool.tile([128, QS], F32, name="recip",
                                           tag="recip", bufs=1)
                        nc.vector.reciprocal_approx_fast(recip, dn_ps)
                        nc.vector.tensor_mul(
                            avt_b[:, h, qs * QS:(qs + 1) * QS], av_ps, recip)

                    # partial out-projection for this query span (both heads
                    # are now done for tokens qs*QS .. (qs+1)*QS)
                    for tloc in range(QS // 128):
                        tch = qs * (QS // 128) + tloc
                        out_sb = opool.tile([128, D], F32, name="out_sb",
                                            tag="out_sb")
                        for dsp in range(D // 512):
                            ops = ps.tile([128, 512], F32, name="ops", tag="pj",
                                          bufs=2)
                            for h in range(HP):
                                nc.tensor.matmul(
                                    ops, avt_b[:, h, tch * 128:(tch + 1) * 128],
                                    wo_sb[:, h, dsp * 512:(dsp + 1) * 512],
                                    start=(h == 0), stop=(h == HP - 1))
                            nc.vector.tensor_copy(
                                out_sb[:, dsp * 512:(dsp + 1) * 512], ops)
                        nc.sync.dma_start(
                            out=out[b * S + tch * 128:b * S + (tch + 1) * 128, :],
                            in_=out_sb)
    nc.compile()
    _BUILT = nc
    return nc


def _install_trace_hooks():
    import types
    try:
        import antenv.axon_hooks  # noqa: F401
        return True
    except ImportError:
        pass
    try:
        from trn_agent_boot.trn_boot import _ntff_profile_via_ctypes
        hook = _ntff_profile_via_ctypes('/opt/axon/libaxon_pjrt.so')
        if hook is None:
            return False
        m = types.ModuleType('antenv.axon_hooks')
        m.get_axon_ntff_profile_hook = lambda: hook
        sys.modules['antenv.axon_hooks'] = m
        from concourse import bass_utils
        bass_utils.upload_artifacts = lambda tmpdir: "local://" + tmpdir
        return True
    except Exception:
        return False


def kernel(x, wq, bq, wk, bk, wv, bv, wo, bo):
    global LAST_EXEC_NS
    from concourse.bass_utils import run_bass_kernel_spmd

    x = np.asarray(x, dtype=np.float32)
    wq = np.asarray(wq, dtype=np.float32)
    bq = np.asarray(bq, dtype=np.float32)
    wk = np.asarray(wk, dtype=np.float32)
    bk = np.asarray(bk, dtype=np.float32)
    wv = np.asarray(wv, dtype=np.float32)
    bv = np.asarray(bv, dtype=np.float32)
    wo = np.asarray(wo, dtype=np.float32)
    bo = np.asarray(bo, dtype=np.float32)

    xt = _round_tf32(x.reshape(TOK, D).T)
    ones = np.ones((128, 128), dtype=np.float32)
    in_maps = []
    for i in range(NCORES):
        sl = slice(i * DC, (i + 1) * DC)
        in_maps.append({
            "xt": xt,
            "wq": _round_tf32(wq[:, sl]),
            "wk": _round_tf32(wk[:, sl]),
            "wv": _round_tf32(wv[:, sl]),
            "wo": _round_tf32(wo[sl, :]),
            "bq2": np.ascontiguousarray(bq[sl].reshape(HP, HD).T),
            "bk2": np.ascontiguousarray(bk[sl].reshape(HP, HD).T),
            "ones": ones,
        })

    trace = bool(os.environ.get("KERNEL_TRACE"))
    if trace:
        trace = _install_trace_hooks()

    nc = _build()
    res = run_bass_kernel_spmd(nc, in_maps, list(range(NCORES)), trace=trace)
    LAST_EXEC_NS = res.exec_time_ns

    total = np.zeros((TOK, D), dtype=np.float32)
    for r in res.results:
        total += r["out"]
    # V-bias folds into a constant row: softmax rows sum to 1, so
    # attention(V + 1*bv^T) = attention(V) + 1*bv^T, and (bv @ wo) adds to bo.
    total += bo + bv @ wo
    return total.reshape(B, S, D)
